# revision 6
# baseline (speedup 1.0000x reference)
"""BitDelta linear on 8 TRN2 NeuronCores.

C[b,s,o] = sum_i X[b,s,i] * (base[o,i] + (2*signs[o,i]-1)*scales[o])

Sharding: 2 token-groups x 4 feature-groups (tensor-parallel on out_features
per the hint, plus a token split to halve per-core X traffic). Each core:
  X_c [8192, 4096] f32, base_c/signs_c [1024, 4096], scales_c [1024]
  -> out_c [8192, 1024] f32
Per core the weight delta is applied on DVE, W is transposed k-major via
identity-matmul on the PE (bf16), kept SBUF-resident; X is streamed per
128-token tile, cast to bf16, transposed via identity-matmul, and
MM-accumulated over k into PSUM (fp32).
"""

import sys

sys.path.insert(0, "/opt/trn_rl_repo")

from contextlib import ExitStack

import numpy as np

import concourse.bass as bass
import concourse.tile as tile
from concourse import bacc, mybir
from concourse.alu_op_type import AluOpType
from concourse.masks import make_identity

F32 = mybir.dt.float32
BF16 = mybir.dt.bfloat16
I32 = mybir.dt.int32
P = 128

B, S, IN, OUT = 8, 2048, 4096, 4096
T = B * S
TG, FG = 2, 4
T_C, F_C = T // TG, OUT // FG
N_CORES = 8


def build_bass(T_c=T_C, F_c=F_C, K=IN, cast_engine="gpsimd", repeat=1,
               dma_cast=False, xtp_bufs=2, stbf_bufs=2, st32_bufs=3,
               xbf_bufs=3):
    nc = bacc.Bacc("TRN2", target_bir_lowering=False, debug=False,
                   enable_asserts=False, num_devices=1)

    x_ap = nc.dram_tensor("x", [T_c, K], F32, kind="ExternalInput").ap()
    base_ap = nc.dram_tensor("base", [F_c, K], F32, kind="ExternalInput").ap()
    signs_ap = nc.dram_tensor("signs", [F_c, K], I32, kind="ExternalInput").ap()
    scales_ap = nc.dram_tensor("scales", [F_c], F32, kind="ExternalInput").ap()
    out_ap = nc.dram_tensor("out", [T_c, F_c], F32, kind="ExternalOutput").ap()

    KT = K // P          # k-tiles of 128
    WTILES = F_c // P    # weight row tiles
    TT = T_c // P        # token tiles
    FB = F_c // 512      # 512-wide output feature blocks
    KG = KT // 4         # groups of 4 k-tiles per psum tile

    with tile.TileContext(nc) as tc:
        with ExitStack() as ctx:
            const = ctx.enter_context(tc.tile_pool(name="const", bufs=1))
            st32 = ctx.enter_context(tc.tile_pool(name="st32", bufs=st32_bufs))
            stbf = ctx.enter_context(tc.tile_pool(name="stbf", bufs=stbf_bufs))
            xbfp = (ctx.enter_context(tc.tile_pool(name="xbfp", bufs=xbf_bufs))
                    if dma_cast else None)
            xtp = ctx.enter_context(tc.tile_pool(name="xtp", bufs=xtp_bufs))
            wtp = ctx.enter_context(tc.tile_pool(name="wtp", bufs=1))
            outp = ctx.enter_context(tc.tile_pool(name="outp", bufs=2))
            ps_t = ctx.enter_context(tc.tile_pool(name="ps_t", bufs=2, space="PSUM"))
            ps_c = ctx.enter_context(
                tc.tile_pool(name="ps_c", bufs=2 * FB, space="PSUM"))

            if repeat > 1:
                ctx.enter_context(tc.For_i(0, repeat, 1))

            ident = const.tile([P, P], BF16)
            make_identity(nc, ident)

            # scales [F_c] -> SBUF [128, WTILES]; f = wt*128 + p
            sc = const.tile([P, WTILES], F32)
            nc.sync.dma_start(sc[:], scales_ap.rearrange("(a p) -> p a", p=P))
            sc2 = const.tile([P, WTILES], F32)
            nc.vector.tensor_scalar_mul(sc2[:], sc[:], 2.0)

            # resident k-major weights: [128, KT, F_c] bf16
            wt_sb = wtp.tile([P, KT, F_c], BF16)

            # ---- Phase W: prepare + transpose weights ----
            for wt in range(WTILES):
                bt = st32.tile([P, K], F32, tag="st32")
                nc.sync.dma_start(bt[:], base_ap[wt * P:(wt + 1) * P, :])
                st = st32.tile([P, K], I32, tag="st32")
                nc.sync.dma_start(st[:], signs_ap[wt * P:(wt + 1) * P, :])

                # b2 = base - scale (per-partition scalar)
                b2 = st32.tile([P, K], F32, tag="st32")
                nc.vector.tensor_scalar_sub(b2[:], bt[:], sc[:, wt:wt + 1])
                # w = signs * (2*scale) + b2 -> bf16
                wbf = stbf.tile([P, K], BF16, tag="stbf")
                nc.vector.scalar_tensor_tensor(
                    out=wbf[:], in0=st[:], scalar=sc2[:, wt:wt + 1], in1=b2[:],
                    op0=AluOpType.mult, op1=AluOpType.add)

                for kg in range(KG):
                    ps = ps_t.tile([P, 512], F32, tag="ps_t")
                    for j in range(4):
                        kt = kg * 4 + j
                        nc.tensor.matmul(
                            ps[:, j * P:(j + 1) * P],
                            wbf[:, kt * P:(kt + 1) * P], ident[:])
                    nc.any.tensor_copy(
                        out=wt_sb[:, kg * 4:(kg + 1) * 4, wt * P:(wt + 1) * P],
                        in_=ps.rearrange("p (a b) -> p a b", b=P))

            # ---- Phase C: stream tokens, software-pipelined so the
            # transpose id-matmuls for tile t+1 interleave with tile t's
            # compute matmuls (their LDWEIGHTS hide under the 512-col
            # streams). ----
            def load_xbf(t):
                if dma_cast:
                    xbf = xbfp.tile([P, K], BF16, tag="xbf", name="xbf")
                    nc.gpsimd.dma_start(xbf[:], x_ap[t * P:(t + 1) * P, :])
                else:
                    xt32 = st32.tile([P, K], F32, tag="st32", name="xt32")
                    nc.sync.dma_start(xt32[:], x_ap[t * P:(t + 1) * P, :])
                    xbf = stbf.tile([P, K], BF16, tag="stbf", name="xbf")
                    getattr(nc, cast_engine).tensor_copy(out=xbf[:], in_=xt32[:])
                return xbf

            def transpose_group(xbf, xt_sb, kg):
                """id-matmul k-slices 4kg..4kg+3 of xbf into PSUM, copy to
                xt_sb."""
                ps = ps_t.tile([P, 512], F32, tag="ps_t", name="ps")
                for j in range(4):
                    kt = kg * 4 + j
                    nc.tensor.matmul(
                        ps[:, j * P:(j + 1) * P],
                        xbf[:, kt * P:(kt + 1) * P], ident[:])
                nc.any.tensor_copy(
                    out=xt_sb[:, kg * 4:(kg + 1) * 4, :],
                    in_=ps.rearrange("p (a b) -> p a b", b=P))

            xbf_cur = load_xbf(0)
            xt_cur = xtp.tile([P, KT, P], BF16, tag="xt", name="xt_cur")
            for kg in range(KG):
                transpose_group(xbf_cur, xt_cur, kg)

            for t in range(TT):
                if t + 1 < TT:
                    xbf_nxt = load_xbf(t + 1)
                    xt_nxt = xtp.tile([P, KT, P], BF16, tag="xt", name="xt_nxt")

                pcs = [ps_c.tile([P, 512], F32, tag="ps_c", name=f"pc{f}")
                       for f in range(FB)]
                for k in range(KT):
                    for f in range(FB):
                        nc.tensor.matmul(
                            pcs[f][:], xt_cur[:, k, :],
                            wt_sb[:, k, f * 512:(f + 1) * 512],
                            start=(k == 0), stop=(k == KT - 1))
                    # interleave next tile's transposes: one group of 4
                    # id-matmuls every 4th k step
                    if t + 1 < TT and k % 4 == 3:
                        transpose_group(xbf_nxt, xt_nxt, k // 4)

                ot = outp.tile([P, F_c], F32)
                for f in range(FB):
                    nc.any.tensor_copy(
                        out=ot[:, f * 512:(f + 1) * 512], in_=pcs[f][:])
                nc.sync.dma_start(out_ap[t * P:(t + 1) * P, :], ot[:])
                if t + 1 < TT:
                    xbf_cur, xt_cur = xbf_nxt, xt_nxt

    nc.compile()
    return nc


class SpmdRunner:
    """Builds the sharded jitted callable once (mirrors
    concourse.bass2jax.run_bass_via_pjrt's multi-core branch) so repeated
    executions skip re-tracing and reuse the cached NEFF."""

    def __init__(self, nc, n_cores):
        import jax
        from jax.sharding import Mesh, PartitionSpec
        from jax.experimental.shard_map import shard_map
        from concourse.bass2jax import (
            _bass_exec_p, install_neuronx_cc_hook, partition_id_tensor)

        self.jax = jax
        self.PartitionSpec = PartitionSpec
        install_neuronx_cc_hook()
        assert nc.dbg_addr is None
        self.n_cores = n_cores
        partition_name = (
            nc.partition_id_tensor.name if nc.partition_id_tensor else None)
        in_names, out_names, out_avals, zero_outs = [], [], [], []
        for alloc in nc.m.functions[0].allocations:
            if not isinstance(alloc, mybir.MemoryLocationSet):
                continue
            name = alloc.memorylocations[0].name
            if alloc.kind == "ExternalInput":
                if name != partition_name:
                    in_names.append(name)
            elif alloc.kind == "ExternalOutput":
                shape = tuple(alloc.tensor_shape)
                dtype = mybir.dt.np(alloc.dtype)
                out_names.append(name)
                out_avals.append(jax.core.ShapedArray(shape, dtype))
                zero_outs.append(np.zeros(shape, dtype))
        n_params = len(in_names)
        n_outs = len(out_avals)
        full_in_names = list(in_names) + list(out_names)
        if partition_name is not None:
            full_in_names.append(partition_name)
        self.in_names = in_names
        self.out_names = out_names
        self.out_avals = out_avals
        self.zero_outs = zero_outs

        def _body(*args):
            operands = list(args)
            if partition_name is not None:
                operands.append(partition_id_tensor())
            outs = _bass_exec_p.bind(
                *operands,
                out_avals=tuple(out_avals),
                in_names=tuple(full_in_names),
                out_names=tuple(out_names),
                lowering_input_output_aliases=(),
                sim_require_finite=True,
                sim_require_nnan=True,
                nc=nc,
            )
            return tuple(outs)

        devices = jax.devices()[:n_cores]
        assert len(devices) == n_cores, (
            f"need {n_cores} cores, have {len(jax.devices())}")
        mesh = Mesh(np.asarray(devices), ("core",))
        in_specs = (PartitionSpec("core"),) * (n_params + n_outs)
        out_specs = (PartitionSpec("core"),) * n_outs
        donate = tuple(range(n_params, n_params + n_outs))
        self.sharded = jax.jit(
            shard_map(_body, mesh=mesh, in_specs=in_specs,
                      out_specs=out_specs, check_rep=False),
            donate_argnums=donate, keep_unused=True)
        self.mesh = mesh

    def prep_inputs(self, in_maps):
        from jax.sharding import NamedSharding

        sh = NamedSharding(self.mesh, self.PartitionSpec("core"))
        concat = [
            np.concatenate([np.asarray(in_maps[c][name])
                            for c in range(self.n_cores)], axis=0)
            for name in self.in_names
        ]
        out = [self.jax.device_put(a, sh) for a in concat]
        self.jax.block_until_ready(out)
        return out

    def zeros(self):
        import jax.numpy as jnp
        from jax.sharding import NamedSharding

        if not hasattr(self, "_zeros_fn"):
            shardings = tuple(
                NamedSharding(self.mesh, self.PartitionSpec("core"))
                for _ in self.zero_outs)
            shapes = [((self.n_cores * z.shape[0], *z.shape[1:]), z.dtype)
                      for z in self.zero_outs]
            self._zeros_fn = self.jax.jit(
                lambda: tuple(jnp.zeros(s, d) for s, d in shapes),
                out_shardings=shardings)
        out = self._zeros_fn()
        self.jax.block_until_ready(out)
        return list(out)

    def __call__(self, prepped_inputs, zeros=None):
        if zeros is None:
            zeros = self.zeros()
        out_arrs = self.sharded(*prepped_inputs, *zeros)
        self.jax.block_until_ready(out_arrs)
        return out_arrs

    def split_outputs(self, out_arrs):
        return [
            {name: np.asarray(out_arrs[i]).reshape(
                self.n_cores, *self.out_avals[i].shape)[c]
             for i, name in enumerate(self.out_names)}
            for c in range(self.n_cores)
        ]


_CACHE = {}


def _get_runner(repeat=1):
    key = repeat
    if key not in _CACHE:
        nc = build_bass(repeat=repeat)
        _CACHE[key] = SpmdRunner(nc, N_CORES)
    return _CACHE[key]


def _shard_inputs(input, base_weight, delta_signs, delta_scales):
    X = np.ascontiguousarray(
        np.asarray(input, dtype=np.float32).reshape(T, IN))
    base = np.asarray(base_weight, dtype=np.float32)
    signs = np.asarray(delta_signs, dtype=np.int32)
    scales = np.asarray(delta_scales, dtype=np.float32)
    in_maps = []
    for c in range(N_CORES):
        tg, fg = divmod(c, FG)
        in_maps.append({
            "x": X[tg * T_C:(tg + 1) * T_C],
            "base": base[fg * F_C:(fg + 1) * F_C],
            "signs": signs[fg * F_C:(fg + 1) * F_C],
            "scales": scales[fg * F_C:(fg + 1) * F_C],
        })
    return in_maps


def kernel(input, base_weight, delta_signs, delta_scales):
    runner = _get_runner()
    in_maps = _shard_inputs(input, base_weight, delta_signs, delta_scales)
    prepped = runner.prep_inputs(in_maps)
    out_arrs = runner(prepped)
    res = runner.split_outputs(out_arrs)
    C = np.empty((T, OUT), np.float32)
    for c in range(N_CORES):
        tg, fg = divmod(c, FG)
        C[tg * T_C:(tg + 1) * T_C, fg * F_C:(fg + 1) * F_C] = res[c]["out"]
    return C.reshape(B, S, OUT)
